# revision 31
# baseline (speedup 1.0000x reference)
"""Trainium2 Bass kernel for causal self-attention with GQA + RoPE.

Problem: x[2,2048,2048], Wq[2048,2048], Wkv[2048,1024], Wproj[2048,2048],
16 q heads, 4 kv heads, head_dim 128, causal softmax, RoPE.

Sharding: 8 cores <-> (batch b in {0,1}) x (kv group g in {0..3}).
Each core computes its 4 q heads + 1 kv head for one batch, producing a
partial output z_partial[T, C] = y_heads @ Wproj[rows of those heads].
Host sums the 4 partials per batch (the Wproj row-shard allreduce).

On-core layout (contraction dims on SBUF partitions, matmuls at free-dim
512; all DRAM traffic and SBUF residents in bf16, PSUM accumulation f32):
  qT[h] = (Wq_h' x')      [hd=128, T]   (1/sqrt(hd) folded into Wq)
  kT    = (Wk'  x')       [128, T]
  vT    = (Wv'  x')       [128, T] -> PE-transposed to vv [T(j), hd]
  RoPE rotate-half via partition-shifted DVE muls with a sign-folded sin
  table (no PE matmul).
  sT[j-chunk, i-tile] = kT_chunk.T @ qT -> exp on ACT -> e (bf16)
  causal mask on diag strips via DVE affine_select.
  y^T[d, i] += v_chunk.T @ e ; Z via ones.T @ (tree-summed e groups on
  gpsimd: 4-8 e tiles per PE matmul instead of 2)
  y^T *= 1/Z (reciprocal_approx_fast), out z[i,:] = sum_h yT_h.T @ Wp_h
All SBUF streaming pools stay open across phases (no pool-reuse barriers);
only PSUM pools nest per phase.
"""

import sys

for _p in ("/opt/trn_rl_repo",):
    if _p not in sys.path:
        sys.path.insert(0, _p)

import ml_dtypes
import numpy as np

BF16 = ml_dtypes.bfloat16

B, T, C = 2, 2048, 2048
NH, NKV, HD = 16, 4, 128
GH = NH // NKV  # q heads per core = 4
GW = GH * HD  # 512
NCC = C // 128  # 16 contraction chunks
NIT = T // 512  # 4 i-tiles
NJC = T // 128  # 16 j-chunks
NCORES = 8

_CACHE = {}


def _host_tables():
    if "tables" in _CACHE:
        return _CACHE["tables"]
    m = np.arange(HD // 2)
    theta = 10000.0 ** (-2.0 * m / HD)
    fr = np.outer(np.arange(T, dtype=np.float64), theta)  # [T, 64]
    cos = np.cos(fr)
    sin = np.sin(fr)
    cosT = np.ascontiguousarray(np.concatenate([cos, cos], 1).T).astype(BF16)
    # sign-folded, partition-swapped sin table: rows 64:128 hold -sin (they
    # multiply x[64:128] in-base), rows 0:64 hold +sin (they multiply x[0:64])
    sinNT = np.ascontiguousarray(np.concatenate([sin, -sin], 1).T).astype(BF16)
    ones = np.ones((128, 128), dtype=BF16)
    ident = np.eye(128, dtype=BF16)
    # causal mask for the leading 128 cols of a diagonal strip: keep
    # e[jl, il] where il >= jl
    tri = (np.arange(128)[None, :] >= np.arange(128)[:, None]).astype(BF16)
    _CACHE["tables"] = (cosT, sinNT, ones, ident, tri)
    return _CACHE["tables"]


def _build_nc():
    if "nc" in _CACHE:
        return _CACHE["nc"]
    import concourse.bacc as bacc
    import concourse.mybir as mybir
    import concourse.tile as tile

    f32 = mybir.dt.float32
    bf16 = mybir.dt.bfloat16
    Exp = mybir.ActivationFunctionType.Exp
    Copy = mybir.ActivationFunctionType.Copy

    nc = bacc.Bacc("TRN2", debug=False, num_devices=NCORES)

    def din(name, shape, dt=bf16):
        return nc.dram_tensor(name, shape, dt, kind="ExternalInput").ap()

    xT = din("xT", [C, T])
    wq = din("wq", [C, GW])
    # wk/wv arrive pre-shuffled to the SBUF tile layout [p, co, d] so the
    # load is 128 contiguous 4KB descriptors instead of 2048 x 256B
    wk = din("wk", [128, NCC, HD])
    wv = din("wv", [128, NCC, HD])
    wp = din("wp", [GW, C])
    cosT = din("cosT", [HD, T])
    sinNT = din("sinNT", [HD, T])
    identm = din("identm", [128, 128])
    onesm = din("onesm", [128, 128])
    trim = din("trim", [128, 128])
    z = nc.dram_tensor("z", [T, C], bf16, kind="ExternalOutput").ap()

    HB = HD // 2  # 64, rotate-half block

    with tile.TileContext(nc) as tc:
        with tc.tile_pool(name="persist", bufs=1) as persist:
            qT = [
                persist.tile([128, T], bf16, tag=f"qT{h}", name=f"qT{h}")
                for h in range(GH)
            ]
            kT = persist.tile([128, T], bf16, tag="kT", name="kT")
            vT = persist.tile([128, T], bf16, tag="vT", name="vT")
            vv = persist.tile([128, T], bf16, tag="vv", name="vv")
            yT = [
                persist.tile([128, T], bf16, tag=f"yT{h}", name=f"yT{h}")
                for h in range(GH)
            ]
            cos_t = persist.tile([128, T], bf16, tag="cos", name="cos")
            sin_t = persist.tile([128, T], bf16, tag="sin", name="sin")
            ident_t = persist.tile([128, 128], bf16, tag="ident", name="ident")
            ones_t = persist.tile([128, 128], bf16, tag="ones", name="ones")
            tri_t = persist.tile([128, 128], bf16, tag="tri", name="tri")
            wk_t = persist.tile([128, NCC, HD], bf16, tag="wk", name="wk")
            wv_t = persist.tile([128, NCC, HD], bf16, tag="wv", name="wv")
            wq_t = persist.tile([128, NCC, GW], bf16, tag="wqf", name="wqf")
            wp_t = persist.tile([128, GH, C], bf16, tag="wp", name="wp")

            with (
                tc.tile_pool(name="xq", bufs=8) as xq_pool,
                tc.tile_pool(name="rope", bufs=3) as rope_pool,
                tc.tile_pool(name="ep", bufs=6) as e_pool,
                tc.tile_pool(name="esum", bufs=4) as s_pool,
                tc.tile_pool(name="nrm", bufs=2) as n_pool,
                tc.tile_pool(name="zo", bufs=6) as z_pool,
            ):
                # ---- Phase P: projections + rope ----
                with (
                    tc.tile_pool(name="pacc", bufs=1, space="PSUM") as pacc,
                    tc.tile_pool(name="prot", bufs=2, space="PSUM") as prot,
                ):
                    # startup: first chunk's inputs on idle queues so the
                    # first matmul isn't gated by one serial queue
                    xt0 = xq_pool.tile([128, 512], bf16, tag="xt", name="xt0")
                    nc.scalar.dma_start(xt0[:], xT[0:128, 0:512])
                    nc.scalar.dma_start(wq_t[:, 0, :], wq[0:128, :])
                    # small tables + kv weights on the gpsimd queue
                    nc.gpsimd.dma_start(wk_t[:, 0:2], wk[:, 0:2])
                    nc.gpsimd.dma_start(wv_t[:, 0:2], wv[:, 0:2])
                    nc.gpsimd.dma_start(wk_t[:, 2:NCC], wk[:, 2:NCC])
                    nc.gpsimd.dma_start(wv_t[:, 2:NCC], wv[:, 2:NCC])
                    nc.gpsimd.dma_start(ident_t[:], identm)
                    nc.gpsimd.dma_start(ones_t[:], onesm)
                    nc.gpsimd.dma_start(tri_t[:], trim)
                    # rope tables on the scalar queue
                    nc.scalar.dma_start(cos_t[:], cosT)
                    nc.scalar.dma_start(sin_t[:], sinNT)
                    for it in range(NIT):
                        I0 = it * 512
                        if it == 1:
                            # wp is only needed in phase O; load during the
                            # DMA-light it=1 window on the idle gpsimd queue
                            nc.gpsimd.dma_start(
                                wp_t[:], wp.rearrange("(hc p) c -> p hc c", p=128)
                            )
                        ps_q = [
                            pacc.tile([128, 512], f32, tag=f"psq{h}", name=f"psq{h}")
                            for h in range(GH)
                        ]
                        ps_k = pacc.tile([128, 512], f32, tag="psk", name="psk")
                        ps_v = pacc.tile([128, 512], f32, tag="psv", name="psv")
                        for c in range(NCC):
                            if it == 0 and c == 0:
                                xt = xt0
                            else:
                                xt = xq_pool.tile([128, 512], bf16, tag="xt", name="xt")
                                nc.sync.dma_start(
                                    xt[:], xT[c * 128 : (c + 1) * 128, I0 : I0 + 512]
                                )
                                if it == 0:
                                    nc.sync.dma_start(
                                        wq_t[:, c, :], wq[c * 128 : (c + 1) * 128, :]
                                    )
                            st = c == 0
                            sp = c == NCC - 1
                            for h in range(GH):
                                nc.tensor.matmul(
                                    ps_q[h][:],
                                    wq_t[:, c, h * HD : (h + 1) * HD],
                                    xt[:],
                                    start=st,
                                    stop=sp,
                                )
                            nc.tensor.matmul(
                                ps_k[:], wk_t[:, c], xt[:], start=st, stop=sp
                            )
                            nc.tensor.matmul(
                                ps_v[:], wv_t[:, c], xt[:], start=st, stop=sp
                            )
                        # RoPE on DVE: dst = ps*cos + shift(ps)*sinN, where
                        # sinN rows 0:64 are -sin.  k first mid-P (gates the
                        # next i-tile's psk bank); q0 first on the last tile
                        # (gates phase A's h=0).
                        rope_jobs = [(ps_k, kT)] + [(ps_q[h], qT[h]) for h in range(GH)]
                        if it == NIT - 1:
                            rope_jobs = [rope_jobs[1], rope_jobs[0]] + rope_jobs[2:]
                        for rj, (ps, dst) in enumerate(rope_jobs):
                            plain = rope_pool.tile(
                                [128, 512], bf16, tag="plain", name="plain"
                            )
                            nc.scalar.activation(plain[:], ps[:], Copy)
                            t1 = rope_pool.tile([128, 512], bf16, tag="t1", name="t1")
                            nc.vector.tensor_mul(
                                out=t1[:], in0=plain[:], in1=cos_t[:, I0 : I0 + 512]
                            )
                            t2 = rope_pool.tile([128, 512], bf16, tag="t2", name="t2")
                            nc.vector.tensor_mul(
                                out=t2[0:HB, :],
                                in0=plain[HB:128, :],
                                in1=sin_t[HB:128, I0 : I0 + 512],
                            )
                            nc.vector.tensor_mul(
                                out=t2[HB:128, :],
                                in0=plain[0:HB, :],
                                in1=sin_t[0:HB, I0 : I0 + 512],
                            )
                            nc.vector.tensor_add(
                                out=dst[:, I0 : I0 + 512], in0=t1[:], in1=t2[:]
                            )
                            if (it < NIT - 1 and rj == 0) or (it == NIT - 1 and rj == 1):
                                # v: copy out of psum, transpose to [j, d]
                                nc.scalar.activation(
                                    vT[:, I0 : I0 + 512], ps_v[:], Copy
                                )
                                for jc in range(4 * it, 4 * (it + 1)):
                                    pst = prot.tile(
                                        [128, 512], bf16, tag="psrot", name="pst"
                                    )
                                    nc.tensor.transpose(
                                        pst[:, :128],
                                        vT[:, jc * 128 : (jc + 1) * 128],
                                        ident_t[:],
                                    )
                                    nc.vector.tensor_copy(
                                        out=vv[:, jc * 128 : (jc + 1) * 128],
                                        in_=pst[:, :128],
                                    )

                # ---- Phase A: attention ----
                # scores/exp run on PAIRS of j-chunks: one [128,1024] psum
                # tile (2 banks) per pair, ONE exp per pair (the ACT engine
                # is the per-element bottleneck in this phase).  The second
                # strip of a pair always sits at free-offset 512.
                with (
                    tc.tile_pool(name="pss", bufs=2, space="PSUM") as pss_pool,
                    tc.tile_pool(name="pyz", bufs=2, space="PSUM") as pyz_pool,
                ):
                    for h in range(GH):
                        for it in range(NIT):
                            I0 = it * 512
                            nj = 4 * (it + 1)
                            ng = it + 1  # groups of 4 j-chunks; diag is last
                            ps_y = pyz_pool.tile([128, 512], f32, tag="psy", name="psy")
                            ps_z = pyz_pool.tile([128, 512], f32, tag="psz", name="psz")
                            n_z = it + 2  # one z matmul per off-diag group,
                            zi = 0  # two for the diag group
                            zjobs = []  # (esum_tile, offset, width) pending

                            def _flush_z():
                                nonlocal zi
                                for st_, w0_, w_ in zjobs:
                                    nc.tensor.matmul(
                                        ps_z[:, w0_ : w0_ + w_],
                                        ones_t[:],
                                        st_[:, :w_],
                                        start=(zi == 0),
                                        stop=(zi == n_z - 1),
                                    )
                                    zi += 1
                                zjobs.clear()

                            for g in range(ng):
                                diag = g == it
                                es = [None] * 2  # the two pair e-tiles
                                _flush_z()
                                for pr in range(2):  # pair within group
                                    ps_s = pss_pool.tile(
                                        [128, 1024], f32, tag="pss", name="pss"
                                    )
                                    e = e_pool.tile(
                                        [128, 1024], bf16, tag="e", name="e"
                                    )
                                    ws = [0, 0]
                                    for u2 in range(2):
                                        u = 2 * pr + u2
                                        jc = 4 * g + u
                                        w0 = u * 128 if diag else 0
                                        w = 512 - w0
                                        ws[u2] = w
                                        nc.tensor.matmul(
                                            ps_s[:, u2 * 512 : u2 * 512 + w],
                                            kT[:, jc * 128 : (jc + 1) * 128],
                                            qT[h][:, I0 + w0 : I0 + 512],
                                            start=True,
                                            stop=True,
                                        )
                                    wend = 512 + ws[1]
                                    nc.scalar.activation(
                                        e[:, :wend], ps_s[:, :wend], Exp
                                    )
                                    if diag:
                                        # zero e[jl, il'] where il' < jl on
                                        # the leading 128 cols of each strip
                                        nc.gpsimd.tensor_mul(
                                            out=e[:, 0:128],
                                            in0=e[:, 0:128],
                                            in1=tri_t[:],
                                        )
                                        nc.gpsimd.tensor_mul(
                                            out=e[:, 512:640],
                                            in0=e[:, 512:640],
                                            in1=tri_t[:],
                                        )
                                    for u2 in range(2):
                                        u = 2 * pr + u2
                                        jc = 4 * g + u
                                        w0 = u * 128 if diag else 0
                                        nc.tensor.matmul(
                                            ps_y[:, w0:512],
                                            vv[:, jc * 128 : (jc + 1) * 128],
                                            e[:, u2 * 512 : u2 * 512 + ws[u2]],
                                            start=(jc == 0),
                                            stop=(jc == nj - 1),
                                        )
                                    es[pr] = e
                                # pair-sum e halves on DVE (frees the e pool
                                # quickly); quad-combine off-diag groups on
                                # gpsimd so Z costs ~1 PE stream per group
                                if diag:
                                    # strip widths 512,384,256,128 at i-tile
                                    # offsets 0,128,256,384
                                    s01 = s_pool.tile(
                                        [128, 512], bf16, tag="ta", name="s01"
                                    )
                                    nc.vector.tensor_copy(
                                        out=s01[:, 0:128], in_=es[0][:, 0:128]
                                    )
                                    nc.vector.tensor_add(
                                        out=s01[:, 128:512],
                                        in0=es[0][:, 128:512],
                                        in1=es[0][:, 512:896],
                                    )
                                    s23 = s_pool.tile(
                                        [128, 256], bf16, tag="tb", name="s23"
                                    )
                                    nc.vector.tensor_copy(
                                        out=s23[:, 0:128], in_=es[1][:, 0:128]
                                    )
                                    nc.vector.tensor_add(
                                        out=s23[:, 128:256],
                                        in0=es[1][:, 128:256],
                                        in1=es[1][:, 512:640],
                                    )
                                    zjobs.append((s01, 0, 512))
                                    zjobs.append((s23, 256, 256))
                                else:
                                    s01 = s_pool.tile(
                                        [128, 512], bf16, tag="ta", name="s01"
                                    )
                                    nc.vector.tensor_add(
                                        out=s01[:],
                                        in0=es[0][:, 0:512],
                                        in1=es[0][:, 512:1024],
                                    )
                                    s23 = s_pool.tile(
                                        [128, 512], bf16, tag="tb", name="s23"
                                    )
                                    nc.vector.tensor_add(
                                        out=s23[:],
                                        in0=es[1][:, 0:512],
                                        in1=es[1][:, 512:1024],
                                    )
                                    q4 = s_pool.tile(
                                        [128, 512], bf16, tag="tc", name="q4"
                                    )
                                    nc.gpsimd.tensor_add(
                                        out=q4[:], in0=s01[:], in1=s23[:]
                                    )
                                    zjobs.append((q4, 0, 512))
                            _flush_z()
                            rz = n_pool.tile([128, 512], f32, tag="rz", name="rz")
                            nc.vector.reciprocal_approx_fast(out=rz[:], in_=ps_z[:])
                            nc.vector.tensor_mul(
                                out=yT[h][:, I0 : I0 + 512], in0=ps_y[:], in1=rz[:]
                            )

                # ---- Phase O: output projection ----
                with tc.tile_pool(name="po", bufs=4, space="PSUM") as po_pool:
                    qrot = [nc.sync, nc.gpsimd, nc.scalar, nc.sync]
                    crot = [nc.scalar, nc.vector]
                    for ic in range(T // 128):
                        for ct in range(C // 512):
                            ps_o = po_pool.tile([128, 512], f32, tag="pso", name="pso")
                            for hc in range(GH):
                                nc.tensor.matmul(
                                    ps_o[:],
                                    yT[hc][:, ic * 128 : (ic + 1) * 128],
                                    wp_t[:, hc, ct * 512 : (ct + 1) * 512],
                                    start=(hc == 0),
                                    stop=(hc == GH - 1),
                                )
                            zr = z_pool.tile([128, 512], bf16, tag="zr", name="zr")
                            eng = crot[(ic * 4 + ct) % 2]
                            if eng is nc.scalar:
                                nc.scalar.activation(zr[:], ps_o[:], Copy)
                            else:
                                eng.tensor_copy(out=zr[:], in_=ps_o[:])
                            qrot[ct].dma_start(
                                z[
                                    ic * 128 : (ic + 1) * 128,
                                    ct * 512 : (ct + 1) * 512,
                                ],
                                zr[:],
                            )

    nc.compile()
    _CACHE["nc"] = nc
    return nc


def _in_maps(x, Wq, Wkv, Wproj):
    cosT, sinNT, ones, ident, tri = _host_tables()
    s = 1.0 / np.sqrt(HD)
    xTs = [np.ascontiguousarray(x[b].T).astype(BF16) for b in range(B)]
    maps = []
    for core in range(NCORES):
        b, g = divmod(core, NKV)
        maps.append(
            {
                "xT": xTs[b],
                "wq": np.ascontiguousarray(Wq[:, g * GW : (g + 1) * GW] * s).astype(
                    BF16
                ),
                # pre-shuffled to SBUF layout [p, co, d]
                "wk": np.ascontiguousarray(
                    Wkv[:, g * HD : (g + 1) * HD]
                    .reshape(NCC, 128, HD)
                    .transpose(1, 0, 2)
                ).astype(BF16),
                "wv": np.ascontiguousarray(
                    Wkv[:, NKV * HD + g * HD : NKV * HD + (g + 1) * HD]
                    .reshape(NCC, 128, HD)
                    .transpose(1, 0, 2)
                ).astype(BF16),
                "wp": np.ascontiguousarray(Wproj[g * GW : (g + 1) * GW, :]).astype(
                    BF16
                ),
                "cosT": cosT,
                "sinNT": sinNT,
                "identm": ident,
                "onesm": ones,
                "trim": tri,
            }
        )
    return maps


def _run(inputs, trace=False, trace_kwargs=None):
    from concourse.bass_utils import run_bass_kernel_spmd

    nc = _build_nc()
    maps = _in_maps(
        np.asarray(inputs["x"], dtype=np.float32),
        np.asarray(inputs["Wq"], dtype=np.float32),
        np.asarray(inputs["Wkv"], dtype=np.float32),
        np.asarray(inputs["Wproj"], dtype=np.float32),
    )
    res = run_bass_kernel_spmd(
        nc, maps, list(range(NCORES)), trace=trace, **(trace_kwargs or {})
    )
    out = np.zeros((B, T, C), dtype=np.float32)
    for core in range(NCORES):
        b = core // NKV
        out[b] += np.asarray(res.results[core]["z"], dtype=np.float32)
    return out, res


def kernel(x, Wq, Wkv, Wproj):
    out, _ = _run({"x": x, "Wq": Wq, "Wkv": Wkv, "Wproj": Wproj}, trace=False)
    return out


# revision 34
# speedup vs baseline: 1.0504x; 1.0504x over previous
"""Trainium2 Bass kernel for causal self-attention with GQA + RoPE.

Problem: x[2,2048,2048], Wq[2048,2048], Wkv[2048,1024], Wproj[2048,2048],
16 q heads, 4 kv heads, head_dim 128, causal softmax, RoPE.

Sharding: 8 cores <-> (batch b in {0,1}) x (kv group g in {0..3}).
Each core computes its 4 q heads + 1 kv head for one batch, producing a
partial output z_partial[T, C] = y_heads @ Wproj[rows of those heads].
Host sums the 4 partials per batch (the Wproj row-shard allreduce).

On-core layout (contraction dims on SBUF partitions, matmuls at free-dim
512; all DRAM traffic and SBUF residents in bf16, PSUM accumulation f32):
  qT[h] = (Wq_h' x')      [hd=128, T]   (1/sqrt(hd) folded into Wq)
  kT    = (Wk'  x')       [128, T]
  vT    = (Wv'  x')       [128, T] -> PE-transposed to vv [T(j), hd]
  RoPE rotate-half via partition-shifted DVE muls with a sign-folded sin
  table (no PE matmul).
  sT[j-chunk, i-tile] = kT_chunk.T @ qT -> exp on ACT -> e (bf16)
  causal mask on diag strips via DVE affine_select.
  y^T[d, i] += v_chunk.T @ e ; Z via ones.T @ (tree-summed e groups on
  gpsimd: 4-8 e tiles per PE matmul instead of 2)
  y^T *= 1/Z (reciprocal_approx_fast), out z[i,:] = sum_h yT_h.T @ Wp_h
All SBUF streaming pools stay open across phases (no pool-reuse barriers);
only PSUM pools nest per phase.
"""

import sys

for _p in ("/opt/trn_rl_repo",):
    if _p not in sys.path:
        sys.path.insert(0, _p)

import ml_dtypes
import numpy as np

BF16 = ml_dtypes.bfloat16

B, T, C = 2, 2048, 2048
NH, NKV, HD = 16, 4, 128
GH = NH // NKV  # q heads per core = 4
GW = GH * HD  # 512
NCC = C // 128  # 16 contraction chunks
NIT = T // 512  # 4 i-tiles
NJC = T // 128  # 16 j-chunks
NCORES = 8

_CACHE = {}


def _host_tables():
    if "tables" in _CACHE:
        return _CACHE["tables"]
    m = np.arange(HD // 2)
    theta = 10000.0 ** (-2.0 * m / HD)
    fr = np.outer(np.arange(T, dtype=np.float64), theta)  # [T, 64]
    cos = np.cos(fr)
    sin = np.sin(fr)
    cosT = np.ascontiguousarray(np.concatenate([cos, cos], 1).T).astype(BF16)
    # sign-folded, partition-swapped sin table: rows 64:128 hold -sin (they
    # multiply x[64:128] in-base), rows 0:64 hold +sin (they multiply x[0:64])
    sinNT = np.ascontiguousarray(np.concatenate([sin, -sin], 1).T).astype(BF16)
    ones = np.ones((128, 128), dtype=BF16)
    ident = np.eye(128, dtype=BF16)
    # causal mask for the leading 128 cols of a diagonal strip: keep
    # e[jl, il] where il >= jl
    tri = (np.arange(128)[None, :] >= np.arange(128)[:, None]).astype(BF16)
    _CACHE["tables"] = (cosT, sinNT, ones, ident, tri)
    return _CACHE["tables"]


def _build_nc():
    if "nc" in _CACHE:
        return _CACHE["nc"]
    import concourse.bacc as bacc
    import concourse.mybir as mybir
    import concourse.tile as tile

    f32 = mybir.dt.float32
    bf16 = mybir.dt.bfloat16
    Exp = mybir.ActivationFunctionType.Exp
    Copy = mybir.ActivationFunctionType.Copy

    nc = bacc.Bacc("TRN2", debug=False, num_devices=NCORES)

    def din(name, shape, dt=bf16):
        return nc.dram_tensor(name, shape, dt, kind="ExternalInput").ap()

    xT = din("xT", [C, T])
    wq = din("wq", [C, GW])
    # wk/wv arrive pre-shuffled to the SBUF tile layout [p, co, d] so the
    # load is 128 contiguous 4KB descriptors instead of 2048 x 256B
    wk = din("wk", [128, NCC, HD])
    wv = din("wv", [128, NCC, HD])
    wp = din("wp", [GW, C])
    cosT = din("cosT", [HD, T])
    sinNT = din("sinNT", [HD, T])
    identm = din("identm", [128, 128])
    onesm = din("onesm", [128, 128])
    trim = din("trim", [128, 128])
    z = nc.dram_tensor("z", [T, C], bf16, kind="ExternalOutput").ap()

    HB = HD // 2  # 64, rotate-half block

    with tile.TileContext(nc) as tc:
        with tc.tile_pool(name="persist", bufs=1) as persist:
            qT = [
                persist.tile([128, T], bf16, tag=f"qT{h}", name=f"qT{h}")
                for h in range(GH)
            ]
            kT = persist.tile([128, T], bf16, tag="kT", name="kT")
            vT = persist.tile([128, T], bf16, tag="vT", name="vT")
            vv = persist.tile([128, T], bf16, tag="vv", name="vv")
            yT = [
                persist.tile([128, T], bf16, tag=f"yT{h}", name=f"yT{h}")
                for h in range(GH)
            ]
            cos_t = persist.tile([128, T], bf16, tag="cos", name="cos")
            sin_t = persist.tile([128, T], bf16, tag="sin", name="sin")
            ident_t = persist.tile([128, 128], bf16, tag="ident", name="ident")
            ones_t = persist.tile([128, 128], bf16, tag="ones", name="ones")
            tri_t = persist.tile([128, 128], bf16, tag="tri", name="tri")
            wk_t = persist.tile([128, NCC, HD], bf16, tag="wk", name="wk")
            wv_t = persist.tile([128, NCC, HD], bf16, tag="wv", name="wv")
            wq_t = persist.tile([128, NCC, GW], bf16, tag="wqf", name="wqf")
            wp_t = persist.tile([128, GH, C], bf16, tag="wp", name="wp")

            with (
                tc.tile_pool(name="xq", bufs=8) as xq_pool,
                tc.tile_pool(name="rope", bufs=3) as rope_pool,
                tc.tile_pool(name="ep", bufs=6) as e_pool,
                tc.tile_pool(name="esum", bufs=4) as s_pool,
                tc.tile_pool(name="nrm", bufs=2) as n_pool,
                tc.tile_pool(name="zo", bufs=6) as z_pool,
            ):
                # ---- Phase P: projections + rope ----
                with (
                    tc.tile_pool(name="pacc", bufs=1, space="PSUM") as pacc,
                    tc.tile_pool(name="prot", bufs=2, space="PSUM") as prot,
                ):
                    # startup: first chunk's inputs on idle queues so the
                    # first matmul isn't gated by one serial queue
                    xt0 = xq_pool.tile([128, 512], bf16, tag="xt", name="xt0")
                    nc.scalar.dma_start(xt0[:], xT[0:128, 0:512])
                    nc.scalar.dma_start(wq_t[:, 0, :], wq[0:128, :])
                    # small tables + first kv weight chunks on the gpsimd
                    # queue; the bulk kv/rope loads are staggered through the
                    # in-order sync stream below so they don't starve the
                    # xt chunk cadence on the shared DMA engines
                    nc.gpsimd.dma_start(wk_t[:, 0:2], wk[:, 0:2])
                    nc.gpsimd.dma_start(wv_t[:, 0:2], wv[:, 0:2])
                    nc.gpsimd.dma_start(ident_t[:], identm)
                    nc.gpsimd.dma_start(ones_t[:], onesm)
                    nc.gpsimd.dma_start(tri_t[:], trim)
                    for it in range(NIT):
                        I0 = it * 512
                        if it == 1:
                            # wp is only needed in phase O; load during the
                            # DMA-light it=1 window on the idle gpsimd queue
                            nc.gpsimd.dma_start(
                                wp_t[:], wp.rearrange("(hc p) c -> p hc c", p=128)
                            )
                        ps_q = [
                            pacc.tile([128, 512], f32, tag=f"psq{h}", name=f"psq{h}")
                            for h in range(GH)
                        ]
                        ps_k = pacc.tile([128, 512], f32, tag="psk", name="psk")
                        ps_v = pacc.tile([128, 512], f32, tag="psv", name="psv")
                        for c in range(NCC):
                            if it == 0 and c == 0:
                                xt = xt0
                            else:
                                xt = xq_pool.tile([128, 512], bf16, tag="xt", name="xt")
                                nc.sync.dma_start(
                                    xt[:], xT[c * 128 : (c + 1) * 128, I0 : I0 + 512]
                                )
                                if it == 0:
                                    nc.sync.dma_start(
                                        wq_t[:, c, :], wq[c * 128 : (c + 1) * 128, :]
                                    )
                                    # staggered bulk loads, each just ahead
                                    # of first use
                                    if c == 1:
                                        nc.sync.dma_start(
                                            wk_t[:, 2:6], wk[:, 2:6]
                                        )
                                        nc.sync.dma_start(
                                            wv_t[:, 2:6], wv[:, 2:6]
                                        )
                                    elif c == 4:
                                        nc.sync.dma_start(
                                            wk_t[:, 6:11], wk[:, 6:11]
                                        )
                                        nc.sync.dma_start(
                                            wv_t[:, 6:11], wv[:, 6:11]
                                        )
                                    elif c == 8:
                                        nc.sync.dma_start(
                                            wk_t[:, 11:NCC], wk[:, 11:NCC]
                                        )
                                        nc.sync.dma_start(
                                            wv_t[:, 11:NCC], wv[:, 11:NCC]
                                        )
                                    elif c == 11:
                                        nc.sync.dma_start(cos_t[:], cosT)
                                    elif c == 13:
                                        nc.sync.dma_start(sin_t[:], sinNT)
                            st = c == 0
                            sp = c == NCC - 1
                            for h in range(GH):
                                nc.tensor.matmul(
                                    ps_q[h][:],
                                    wq_t[:, c, h * HD : (h + 1) * HD],
                                    xt[:],
                                    start=st,
                                    stop=sp,
                                )
                            nc.tensor.matmul(
                                ps_k[:], wk_t[:, c], xt[:], start=st, stop=sp
                            )
                            nc.tensor.matmul(
                                ps_v[:], wv_t[:, c], xt[:], start=st, stop=sp
                            )
                        # RoPE on DVE: dst = ps*cos + shift(ps)*sinN, where
                        # sinN rows 0:64 are -sin.  k first mid-P (gates the
                        # next i-tile's psk bank); q0 first on the last tile
                        # (gates phase A's h=0).
                        rope_jobs = [(ps_k, kT)] + [(ps_q[h], qT[h]) for h in range(GH)]
                        if it == NIT - 1:
                            rope_jobs = [rope_jobs[1], rope_jobs[0]] + rope_jobs[2:]
                        for rj, (ps, dst) in enumerate(rope_jobs):
                            plain = rope_pool.tile(
                                [128, 512], bf16, tag="plain", name="plain"
                            )
                            nc.scalar.activation(plain[:], ps[:], Copy)
                            t1 = rope_pool.tile([128, 512], bf16, tag="t1", name="t1")
                            nc.vector.tensor_mul(
                                out=t1[:], in0=plain[:], in1=cos_t[:, I0 : I0 + 512]
                            )
                            t2 = rope_pool.tile([128, 512], bf16, tag="t2", name="t2")
                            nc.vector.tensor_mul(
                                out=t2[0:HB, :],
                                in0=plain[HB:128, :],
                                in1=sin_t[HB:128, I0 : I0 + 512],
                            )
                            nc.vector.tensor_mul(
                                out=t2[HB:128, :],
                                in0=plain[0:HB, :],
                                in1=sin_t[0:HB, I0 : I0 + 512],
                            )
                            nc.vector.tensor_add(
                                out=dst[:, I0 : I0 + 512], in0=t1[:], in1=t2[:]
                            )
                            if (it < NIT - 1 and rj == 0) or (it == NIT - 1 and rj == 1):
                                # v: copy out of psum, transpose to [j, d]
                                nc.scalar.activation(
                                    vT[:, I0 : I0 + 512], ps_v[:], Copy
                                )
                                for jc in range(4 * it, 4 * (it + 1)):
                                    pst = prot.tile(
                                        [128, 512], bf16, tag="psrot", name="pst"
                                    )
                                    nc.tensor.transpose(
                                        pst[:, :128],
                                        vT[:, jc * 128 : (jc + 1) * 128],
                                        ident_t[:],
                                    )
                                    nc.vector.tensor_copy(
                                        out=vv[:, jc * 128 : (jc + 1) * 128],
                                        in_=pst[:, :128],
                                    )

                # ---- Phase A: attention ----
                # scores/exp run on PAIRS of j-chunks: one [128,1024] psum
                # tile (2 banks) per pair, ONE exp per pair (the ACT engine
                # is the per-element bottleneck in this phase).  The second
                # strip of a pair always sits at free-offset 512.
                with (
                    tc.tile_pool(name="pss", bufs=3, space="PSUM") as pss_pool,
                    tc.tile_pool(name="pyz", bufs=1, space="PSUM") as pyz_pool,
                ):
                    for h in range(GH):
                        for it in range(NIT):
                            I0 = it * 512
                            nj = 4 * (it + 1)
                            ng = it + 1  # groups of 4 j-chunks; diag is last
                            ps_y = pyz_pool.tile([128, 512], f32, tag="psy", name="psy")
                            ps_z = pyz_pool.tile([128, 512], f32, tag="psz", name="psz")
                            n_z = it + 2  # one z matmul per off-diag group,
                            zi = 0  # two for the diag group
                            zjobs = []  # (esum_tile, offset, width) pending

                            def _flush_z():
                                nonlocal zi
                                for st_, w0_, w_ in zjobs:
                                    nc.tensor.matmul(
                                        ps_z[:, w0_ : w0_ + w_],
                                        ones_t[:],
                                        st_[:, :w_],
                                        start=(zi == 0),
                                        stop=(zi == n_z - 1),
                                    )
                                    zi += 1
                                zjobs.clear()

                            for g in range(ng):
                                diag = g == it
                                es = [None] * 2  # the two pair e-tiles
                                _flush_z()
                                for pr in range(2):  # pair within group
                                    ps_s = pss_pool.tile(
                                        [128, 1024], f32, tag="pss", name="pss"
                                    )
                                    e = e_pool.tile(
                                        [128, 1024], bf16, tag="e", name="e"
                                    )
                                    ws = [0, 0]
                                    for u2 in range(2):
                                        u = 2 * pr + u2
                                        jc = 4 * g + u
                                        w0 = u * 128 if diag else 0
                                        w = 512 - w0
                                        ws[u2] = w
                                        nc.tensor.matmul(
                                            ps_s[:, u2 * 512 : u2 * 512 + w],
                                            kT[:, jc * 128 : (jc + 1) * 128],
                                            qT[h][:, I0 + w0 : I0 + 512],
                                            start=True,
                                            stop=True,
                                        )
                                    wend = 512 + ws[1]
                                    nc.scalar.activation(
                                        e[:, :wend], ps_s[:, :wend], Exp
                                    )
                                    if diag:
                                        # zero e[jl, il'] where il' < jl on
                                        # the leading 128 cols of each strip
                                        nc.gpsimd.tensor_mul(
                                            out=e[:, 0:128],
                                            in0=e[:, 0:128],
                                            in1=tri_t[:],
                                        )
                                        nc.gpsimd.tensor_mul(
                                            out=e[:, 512:640],
                                            in0=e[:, 512:640],
                                            in1=tri_t[:],
                                        )
                                    for u2 in range(2):
                                        u = 2 * pr + u2
                                        jc = 4 * g + u
                                        w0 = u * 128 if diag else 0
                                        nc.tensor.matmul(
                                            ps_y[:, w0:512],
                                            vv[:, jc * 128 : (jc + 1) * 128],
                                            e[:, u2 * 512 : u2 * 512 + ws[u2]],
                                            start=(jc == 0),
                                            stop=(jc == nj - 1),
                                        )
                                    es[pr] = e
                                # pair-sum e halves on DVE (frees the e pool
                                # quickly); quad-combine off-diag groups on
                                # gpsimd so Z costs ~1 PE stream per group
                                if diag:
                                    # strip widths 512,384,256,128 at i-tile
                                    # offsets 0,128,256,384
                                    s01 = s_pool.tile(
                                        [128, 512], bf16, tag="ta", name="s01"
                                    )
                                    nc.vector.tensor_copy(
                                        out=s01[:, 0:128], in_=es[0][:, 0:128]
                                    )
                                    nc.vector.tensor_add(
                                        out=s01[:, 128:512],
                                        in0=es[0][:, 128:512],
                                        in1=es[0][:, 512:896],
                                    )
                                    s23 = s_pool.tile(
                                        [128, 256], bf16, tag="tb", name="s23"
                                    )
                                    nc.vector.tensor_copy(
                                        out=s23[:, 0:128], in_=es[1][:, 0:128]
                                    )
                                    nc.vector.tensor_add(
                                        out=s23[:, 128:256],
                                        in0=es[1][:, 128:256],
                                        in1=es[1][:, 512:640],
                                    )
                                    zjobs.append((s01, 0, 512))
                                    zjobs.append((s23, 256, 256))
                                else:
                                    s01 = s_pool.tile(
                                        [128, 512], bf16, tag="ta", name="s01"
                                    )
                                    nc.vector.tensor_add(
                                        out=s01[:],
                                        in0=es[0][:, 0:512],
                                        in1=es[0][:, 512:1024],
                                    )
                                    s23 = s_pool.tile(
                                        [128, 512], bf16, tag="tb", name="s23"
                                    )
                                    nc.vector.tensor_add(
                                        out=s23[:],
                                        in0=es[1][:, 0:512],
                                        in1=es[1][:, 512:1024],
                                    )
                                    q4 = s_pool.tile(
                                        [128, 512], bf16, tag="tc", name="q4"
                                    )
                                    nc.gpsimd.tensor_add(
                                        out=q4[:], in0=s01[:], in1=s23[:]
                                    )
                                    zjobs.append((q4, 0, 512))
                            _flush_z()
                            rz = n_pool.tile([128, 512], f32, tag="rz", name="rz")
                            nc.vector.reciprocal_approx_fast(out=rz[:], in_=ps_z[:])
                            nc.vector.tensor_mul(
                                out=yT[h][:, I0 : I0 + 512], in0=ps_y[:], in1=rz[:]
                            )

                # ---- Phase O: output projection ----
                with tc.tile_pool(name="po", bufs=4, space="PSUM") as po_pool:
                    qrot = [nc.sync, nc.gpsimd, nc.scalar, nc.sync]
                    crot = [nc.scalar, nc.vector]
                    for ic in range(T // 128):
                        for ct in range(C // 512):
                            ps_o = po_pool.tile([128, 512], f32, tag="pso", name="pso")
                            for hc in range(GH):
                                nc.tensor.matmul(
                                    ps_o[:],
                                    yT[hc][:, ic * 128 : (ic + 1) * 128],
                                    wp_t[:, hc, ct * 512 : (ct + 1) * 512],
                                    start=(hc == 0),
                                    stop=(hc == GH - 1),
                                )
                            zr = z_pool.tile([128, 512], bf16, tag="zr", name="zr")
                            eng = crot[(ic * 4 + ct) % 2]
                            if eng is nc.scalar:
                                nc.scalar.activation(zr[:], ps_o[:], Copy)
                            else:
                                eng.tensor_copy(out=zr[:], in_=ps_o[:])
                            qrot[ct].dma_start(
                                z[
                                    ic * 128 : (ic + 1) * 128,
                                    ct * 512 : (ct + 1) * 512,
                                ],
                                zr[:],
                            )

    nc.compile()
    _CACHE["nc"] = nc
    return nc


def _in_maps(x, Wq, Wkv, Wproj):
    cosT, sinNT, ones, ident, tri = _host_tables()
    s = 1.0 / np.sqrt(HD)
    xTs = [np.ascontiguousarray(x[b].T).astype(BF16) for b in range(B)]
    maps = []
    for core in range(NCORES):
        b, g = divmod(core, NKV)
        maps.append(
            {
                "xT": xTs[b],
                "wq": np.ascontiguousarray(Wq[:, g * GW : (g + 1) * GW] * s).astype(
                    BF16
                ),
                # pre-shuffled to SBUF layout [p, co, d]
                "wk": np.ascontiguousarray(
                    Wkv[:, g * HD : (g + 1) * HD]
                    .reshape(NCC, 128, HD)
                    .transpose(1, 0, 2)
                ).astype(BF16),
                "wv": np.ascontiguousarray(
                    Wkv[:, NKV * HD + g * HD : NKV * HD + (g + 1) * HD]
                    .reshape(NCC, 128, HD)
                    .transpose(1, 0, 2)
                ).astype(BF16),
                "wp": np.ascontiguousarray(Wproj[g * GW : (g + 1) * GW, :]).astype(
                    BF16
                ),
                "cosT": cosT,
                "sinNT": sinNT,
                "identm": ident,
                "onesm": ones,
                "trim": tri,
            }
        )
    return maps


def _run(inputs, trace=False, trace_kwargs=None):
    from concourse.bass_utils import run_bass_kernel_spmd

    nc = _build_nc()
    maps = _in_maps(
        np.asarray(inputs["x"], dtype=np.float32),
        np.asarray(inputs["Wq"], dtype=np.float32),
        np.asarray(inputs["Wkv"], dtype=np.float32),
        np.asarray(inputs["Wproj"], dtype=np.float32),
    )
    res = run_bass_kernel_spmd(
        nc, maps, list(range(NCORES)), trace=trace, **(trace_kwargs or {})
    )
    out = np.zeros((B, T, C), dtype=np.float32)
    for core in range(NCORES):
        b = core // NKV
        out[b] += np.asarray(res.results[core]["z"], dtype=np.float32)
    return out, res


def kernel(x, Wq, Wkv, Wproj):
    out, _ = _run({"x": x, "Wq": Wq, "Wkv": Wkv, "Wproj": Wproj}, trace=False)
    return out


# revision 39
# speedup vs baseline: 1.0602x; 1.0093x over previous
"""Trainium2 Bass kernel for causal self-attention with GQA + RoPE.

Problem: x[2,2048,2048], Wq[2048,2048], Wkv[2048,1024], Wproj[2048,2048],
16 q heads, 4 kv heads, head_dim 128, causal softmax, RoPE.

Sharding: 8 cores <-> (batch b in {0,1}) x (kv group g in {0..3}).
Each core computes its 4 q heads + 1 kv head for one batch, producing a
partial output z_partial[T, C] = y_heads @ Wproj[rows of those heads].
Host sums the 4 partials per batch (the Wproj row-shard allreduce).

On-core layout (contraction dims on SBUF partitions, matmuls at free-dim
512; all DRAM traffic and SBUF residents in bf16, PSUM accumulation f32):
  qT[h] = (Wq_h' x')      [hd=128, T]   (1/sqrt(hd) folded into Wq)
  kT    = (Wk'  x')       [128, T]
  vT    = (Wv'  x')       [128, T] -> PE-transposed to vv [T(j), hd]
  RoPE rotate-half via partition-shifted DVE muls with a sign-folded sin
  table (no PE matmul).
  sT[j-chunk, i-tile] = kT_chunk.T @ qT -> exp on ACT -> e (bf16)
  causal mask on diag strips via DVE affine_select.
  y^T[d, i] += v_chunk.T @ e ; Z via ones.T @ (tree-summed e groups on
  gpsimd: 4-8 e tiles per PE matmul instead of 2)
  y^T *= 1/Z (reciprocal_approx_fast), out z[i,:] = sum_h yT_h.T @ Wp_h
All SBUF streaming pools stay open across phases (no pool-reuse barriers);
only PSUM pools nest per phase.
"""

import sys

for _p in ("/opt/trn_rl_repo",):
    if _p not in sys.path:
        sys.path.insert(0, _p)

import ml_dtypes
import numpy as np

BF16 = ml_dtypes.bfloat16

B, T, C = 2, 2048, 2048
NH, NKV, HD = 16, 4, 128
GH = NH // NKV  # q heads per core = 4
GW = GH * HD  # 512
NCC = C // 128  # 16 contraction chunks
NIT = T // 512  # 4 i-tiles
NJC = T // 128  # 16 j-chunks
NCORES = 8

_CACHE = {}


def _host_tables():
    if "tables" in _CACHE:
        return _CACHE["tables"]
    m = np.arange(HD // 2)
    theta = 10000.0 ** (-2.0 * m / HD)
    fr = np.outer(np.arange(T, dtype=np.float64), theta)  # [T, 64]
    cos = np.cos(fr)
    sin = np.sin(fr)
    cosT = np.ascontiguousarray(np.concatenate([cos, cos], 1).T).astype(BF16)
    # sign-folded, partition-swapped sin table: rows 64:128 hold -sin (they
    # multiply x[64:128] in-base), rows 0:64 hold +sin (they multiply x[0:64])
    sinNT = np.ascontiguousarray(np.concatenate([sin, -sin], 1).T).astype(BF16)
    ones = np.ones((128, 128), dtype=BF16)
    ident = np.eye(128, dtype=BF16)
    # causal mask for the leading 128 cols of a diagonal strip: keep
    # e[jl, il] where il >= jl
    tri = (np.arange(128)[None, :] >= np.arange(128)[:, None]).astype(BF16)
    _CACHE["tables"] = (cosT, sinNT, ones, ident, tri)
    return _CACHE["tables"]


def _build_nc():
    if "nc" in _CACHE:
        return _CACHE["nc"]
    import concourse.bacc as bacc
    import concourse.mybir as mybir
    import concourse.tile as tile

    f32 = mybir.dt.float32
    bf16 = mybir.dt.bfloat16
    Exp = mybir.ActivationFunctionType.Exp
    Copy = mybir.ActivationFunctionType.Copy

    nc = bacc.Bacc("TRN2", debug=False, num_devices=NCORES)

    def din(name, shape, dt=bf16):
        return nc.dram_tensor(name, shape, dt, kind="ExternalInput").ap()

    xT = din("xT", [C, T])
    wq = din("wq", [C, GW])
    # wk/wv arrive pre-shuffled to the SBUF tile layout [p, co, d] so the
    # load is 128 contiguous 4KB descriptors instead of 2048 x 256B
    wk = din("wk", [128, NCC, HD])
    wv = din("wv", [128, NCC, HD])
    wp = din("wp", [GW, C])
    cosT = din("cosT", [HD, T])
    sinNT = din("sinNT", [HD, T])
    identm = din("identm", [128, 128])
    onesm = din("onesm", [128, 128])
    trim = din("trim", [128, 128])
    z = nc.dram_tensor("z", [T, C], bf16, kind="ExternalOutput").ap()

    HB = HD // 2  # 64, rotate-half block

    with tile.TileContext(nc) as tc:
        with tc.tile_pool(name="persist", bufs=1) as persist:
            qT = [
                persist.tile([128, T], bf16, tag=f"qT{h}", name=f"qT{h}")
                for h in range(GH)
            ]
            kT = persist.tile([128, T], bf16, tag="kT", name="kT")
            vT = persist.tile([128, T], bf16, tag="vT", name="vT")
            vv = persist.tile([128, T], bf16, tag="vv", name="vv")
            yT = [
                persist.tile([128, T], bf16, tag=f"yT{h}", name=f"yT{h}")
                for h in range(GH)
            ]
            cos_t = persist.tile([128, T], bf16, tag="cos", name="cos")
            sin_t = persist.tile([128, T], bf16, tag="sin", name="sin")
            ident_t = persist.tile([128, 128], bf16, tag="ident", name="ident")
            ones_t = persist.tile([128, 128], bf16, tag="ones", name="ones")
            tri_t = persist.tile([128, 128], bf16, tag="tri", name="tri")
            wk_t = persist.tile([128, NCC, HD], bf16, tag="wk", name="wk")
            wv_t = persist.tile([128, NCC, HD], bf16, tag="wv", name="wv")
            wq_t = persist.tile([128, NCC, GW], bf16, tag="wqf", name="wqf")
            wp_t = persist.tile([128, GH, C], bf16, tag="wp", name="wp")

            with (
                tc.tile_pool(name="xq", bufs=8) as xq_pool,
                tc.tile_pool(name="rope", bufs=3) as rope_pool,
                tc.tile_pool(name="ep", bufs=6) as e_pool,
                tc.tile_pool(name="esum", bufs=4) as s_pool,
                tc.tile_pool(name="nrm", bufs=2) as n_pool,
                tc.tile_pool(name="zo", bufs=6) as z_pool,
            ):
                # ---- Phase P: projections + rope ----
                with (
                    tc.tile_pool(name="pacc", bufs=1, space="PSUM") as pacc,
                    tc.tile_pool(name="prot", bufs=2, space="PSUM") as prot,
                ):
                    # startup: first chunk's inputs on idle queues so the
                    # first matmul isn't gated by one serial queue
                    xt0 = xq_pool.tile([128, 512], bf16, tag="xt", name="xt0")
                    nc.scalar.dma_start(xt0[:], xT[0:128, 0:512])
                    nc.scalar.dma_start(wq_t[:, 0, :], wq[0:128, :])
                    # small tables + first kv weight chunks on the gpsimd
                    # queue; the bulk kv/rope loads are staggered through the
                    # in-order sync stream below so they don't starve the
                    # xt chunk cadence on the shared DMA engines
                    nc.gpsimd.dma_start(wk_t[:, 0:2], wk[:, 0:2])
                    nc.gpsimd.dma_start(wv_t[:, 0:2], wv[:, 0:2])
                    nc.gpsimd.dma_start(ident_t[:], identm)
                    nc.gpsimd.dma_start(ones_t[:], onesm)
                    nc.gpsimd.dma_start(tri_t[:], trim)
                    for it in range(NIT):
                        I0 = it * 512
                        if it == 1:
                            # wp is only needed in phase O; load during the
                            # DMA-light it=1 window on the idle gpsimd queue
                            nc.gpsimd.dma_start(
                                wp_t[:], wp.rearrange("(hc p) c -> p hc c", p=128)
                            )
                        ps_q = [
                            pacc.tile([128, 512], f32, tag=f"psq{h}", name=f"psq{h}")
                            for h in range(GH)
                        ]
                        ps_k = pacc.tile([128, 512], f32, tag="psk", name="psk")
                        ps_v = pacc.tile([128, 512], f32, tag="psv", name="psv")
                        for c in range(NCC):
                            if it == 0 and c == 0:
                                xt = xt0
                            else:
                                # alternate chunks between the sync and
                                # scalar DGE queues: descriptor generation
                                # rate, not wire bandwidth, limits the it=0
                                # chunk cadence
                                dq = nc.sync if c % 2 == 0 else nc.scalar
                                xt = xq_pool.tile([128, 512], bf16, tag="xt", name="xt")
                                dq.dma_start(
                                    xt[:], xT[c * 128 : (c + 1) * 128, I0 : I0 + 512]
                                )
                                if it == 0:
                                    dq.dma_start(
                                        wq_t[:, c, :], wq[c * 128 : (c + 1) * 128, :]
                                    )
                                    # staggered bulk loads, each just ahead
                                    # of first use
                                    if c == 1:
                                        nc.sync.dma_start(
                                            wk_t[:, 2:6], wk[:, 2:6]
                                        )
                                        nc.sync.dma_start(
                                            wv_t[:, 2:6], wv[:, 2:6]
                                        )
                                    elif c == 4:
                                        nc.sync.dma_start(
                                            wk_t[:, 6:11], wk[:, 6:11]
                                        )
                                        nc.sync.dma_start(
                                            wv_t[:, 6:11], wv[:, 6:11]
                                        )
                                    elif c == 8:
                                        nc.sync.dma_start(
                                            wk_t[:, 11:NCC], wk[:, 11:NCC]
                                        )
                                        nc.sync.dma_start(
                                            wv_t[:, 11:NCC], wv[:, 11:NCC]
                                        )
                                    elif c == 11:
                                        nc.sync.dma_start(cos_t[:], cosT)
                                    elif c == 13:
                                        nc.sync.dma_start(sin_t[:], sinNT)
                            st = c == 0
                            sp = c == NCC - 1
                            for h in range(GH):
                                nc.tensor.matmul(
                                    ps_q[h][:],
                                    wq_t[:, c, h * HD : (h + 1) * HD],
                                    xt[:],
                                    start=st,
                                    stop=sp,
                                )
                            nc.tensor.matmul(
                                ps_k[:], wk_t[:, c], xt[:], start=st, stop=sp
                            )
                            nc.tensor.matmul(
                                ps_v[:], wv_t[:, c], xt[:], start=st, stop=sp
                            )
                        # RoPE on DVE: dst = ps*cos + shift(ps)*sinN, where
                        # sinN rows 0:64 are -sin.  k first mid-P (gates the
                        # next i-tile's psk bank); q0 first on the last tile
                        # (gates phase A's h=0).
                        rope_jobs = [(ps_k, kT)] + [(ps_q[h], qT[h]) for h in range(GH)]
                        if it == NIT - 1:
                            rope_jobs = [rope_jobs[1], rope_jobs[0]] + rope_jobs[2:]
                        for rj, (ps, dst) in enumerate(rope_jobs):
                            plain = rope_pool.tile(
                                [128, 512], bf16, tag="plain", name="plain"
                            )
                            nc.scalar.activation(plain[:], ps[:], Copy)
                            t1 = rope_pool.tile([128, 512], bf16, tag="t1", name="t1")
                            nc.vector.tensor_mul(
                                out=t1[:], in0=plain[:], in1=cos_t[:, I0 : I0 + 512]
                            )
                            t2 = rope_pool.tile([128, 512], bf16, tag="t2", name="t2")
                            nc.vector.tensor_mul(
                                out=t2[0:HB, :],
                                in0=plain[HB:128, :],
                                in1=sin_t[HB:128, I0 : I0 + 512],
                            )
                            nc.vector.tensor_mul(
                                out=t2[HB:128, :],
                                in0=plain[0:HB, :],
                                in1=sin_t[0:HB, I0 : I0 + 512],
                            )
                            nc.vector.tensor_add(
                                out=dst[:, I0 : I0 + 512], in0=t1[:], in1=t2[:]
                            )
                            if (it < NIT - 1 and rj == 0) or (it == NIT - 1 and rj == 1):
                                # v: copy out of psum, transpose to [j, d]
                                nc.scalar.activation(
                                    vT[:, I0 : I0 + 512], ps_v[:], Copy
                                )
                                for jc in range(4 * it, 4 * (it + 1)):
                                    pst = prot.tile(
                                        [128, 512], bf16, tag="psrot", name="pst"
                                    )
                                    nc.tensor.transpose(
                                        pst[:, :128],
                                        vT[:, jc * 128 : (jc + 1) * 128],
                                        ident_t[:],
                                    )
                                    nc.vector.tensor_copy(
                                        out=vv[:, jc * 128 : (jc + 1) * 128],
                                        in_=pst[:, :128],
                                    )

                # ---- Phase A: attention ----
                # scores/exp run on PAIRS of j-chunks: one [128,1024] psum
                # tile (2 banks) per pair, ONE exp per pair (the ACT engine
                # is the per-element bottleneck in this phase).  The second
                # strip of a pair always sits at free-offset 512.
                with (
                    tc.tile_pool(name="pss", bufs=3, space="PSUM") as pss_pool,
                    tc.tile_pool(name="pyz", bufs=1, space="PSUM") as pyz_pool,
                ):
                    for h in range(GH):
                        for it in range(NIT):
                            I0 = it * 512
                            nj = 4 * (it + 1)
                            ng = it + 1  # groups of 4 j-chunks; diag is last
                            ps_y = pyz_pool.tile([128, 512], f32, tag="psy", name="psy")
                            ps_z = pyz_pool.tile([128, 512], f32, tag="psz", name="psz")
                            n_z = it + 4  # one z matmul per off-diag group,
                            zi = 0  # four for the diag group's e halves
                            zjobs = []  # (rhs_ap, psz_offset, width) pending

                            def _flush_z():
                                nonlocal zi
                                for rhs_, w0_, w_ in zjobs:
                                    nc.tensor.matmul(
                                        ps_z[:, w0_ : w0_ + w_],
                                        ones_t[:],
                                        rhs_,
                                        start=(zi == 0),
                                        stop=(zi == n_z - 1),
                                    )
                                    zi += 1
                                zjobs.clear()

                            for g in range(ng):
                                diag = g == it
                                es = [None] * 2  # the two pair e-tiles
                                _flush_z()
                                for pr in range(2):  # pair within group
                                    ps_s = pss_pool.tile(
                                        [128, 1024], f32, tag="pss", name="pss"
                                    )
                                    e = e_pool.tile(
                                        [128, 1024], bf16, tag="e", name="e"
                                    )
                                    ws = [0, 0]
                                    for u2 in range(2):
                                        u = 2 * pr + u2
                                        jc = 4 * g + u
                                        w0 = u * 128 if diag else 0
                                        w = 512 - w0
                                        ws[u2] = w
                                        nc.tensor.matmul(
                                            ps_s[:, u2 * 512 : u2 * 512 + w],
                                            kT[:, jc * 128 : (jc + 1) * 128],
                                            qT[h][:, I0 + w0 : I0 + 512],
                                            start=True,
                                            stop=True,
                                        )
                                    wend = 512 + ws[1]
                                    nc.scalar.activation(
                                        e[:, :wend], ps_s[:, :wend], Exp
                                    )
                                    if diag:
                                        # zero e[jl, il'] where il' < jl on
                                        # the leading 128 cols of each strip
                                        nc.gpsimd.tensor_mul(
                                            out=e[:, 0:128],
                                            in0=e[:, 0:128],
                                            in1=tri_t[:],
                                        )
                                        nc.gpsimd.tensor_mul(
                                            out=e[:, 512:640],
                                            in0=e[:, 512:640],
                                            in1=tri_t[:],
                                        )
                                    for u2 in range(2):
                                        u = 2 * pr + u2
                                        jc = 4 * g + u
                                        w0 = u * 128 if diag else 0
                                        nc.tensor.matmul(
                                            ps_y[:, w0:512],
                                            vv[:, jc * 128 : (jc + 1) * 128],
                                            e[:, u2 * 512 : u2 * 512 + ws[u2]],
                                            start=(jc == 0),
                                            stop=(jc == nj - 1),
                                        )
                                    es[pr] = e
                                # pair-sum e halves on DVE (frees the e pool
                                # quickly); quad-combine off-diag groups on
                                # gpsimd so Z costs ~1 PE stream per group.
                                # The diag group feeds its e halves straight
                                # to z matmuls: a shorter dependency chain at
                                # the end of the (h, it) tile beats the PE
                                # rows saved by pre-summing.
                                if diag:
                                    # strip widths 512,384,256,128 at i-tile
                                    # offsets 0,128,256,384
                                    zjobs.append((es[0][:, 0:512], 0, 512))
                                    zjobs.append((es[0][:, 512:896], 128, 384))
                                    zjobs.append((es[1][:, 0:256], 256, 256))
                                    zjobs.append((es[1][:, 512:640], 384, 128))
                                else:
                                    s01 = s_pool.tile(
                                        [128, 512], bf16, tag="ta", name="s01"
                                    )
                                    nc.vector.tensor_add(
                                        out=s01[:],
                                        in0=es[0][:, 0:512],
                                        in1=es[0][:, 512:1024],
                                    )
                                    s23 = s_pool.tile(
                                        [128, 512], bf16, tag="tb", name="s23"
                                    )
                                    nc.vector.tensor_add(
                                        out=s23[:],
                                        in0=es[1][:, 0:512],
                                        in1=es[1][:, 512:1024],
                                    )
                                    q4 = s_pool.tile(
                                        [128, 512], bf16, tag="tc", name="q4"
                                    )
                                    nc.gpsimd.tensor_add(
                                        out=q4[:], in0=s01[:], in1=s23[:]
                                    )
                                    zjobs.append((q4[:], 0, 512))
                            _flush_z()
                            rz = n_pool.tile([128, 512], f32, tag="rz", name="rz")
                            nc.vector.reciprocal_approx_fast(out=rz[:], in_=ps_z[:])
                            nc.vector.tensor_mul(
                                out=yT[h][:, I0 : I0 + 512], in0=ps_y[:], in1=rz[:]
                            )

                # ---- Phase O: output projection ----
                with tc.tile_pool(name="po", bufs=4, space="PSUM") as po_pool:
                    qrot = [nc.sync, nc.gpsimd, nc.scalar, nc.sync]
                    crot = [nc.scalar, nc.vector]
                    for ic in range(T // 128):
                        for ct in range(C // 512):
                            ps_o = po_pool.tile([128, 512], f32, tag="pso", name="pso")
                            for hc in range(GH):
                                nc.tensor.matmul(
                                    ps_o[:],
                                    yT[hc][:, ic * 128 : (ic + 1) * 128],
                                    wp_t[:, hc, ct * 512 : (ct + 1) * 512],
                                    start=(hc == 0),
                                    stop=(hc == GH - 1),
                                )
                            zr = z_pool.tile([128, 512], bf16, tag="zr", name="zr")
                            eng = crot[(ic * 4 + ct) % 2]
                            if eng is nc.scalar:
                                nc.scalar.activation(zr[:], ps_o[:], Copy)
                            else:
                                eng.tensor_copy(out=zr[:], in_=ps_o[:])
                            qrot[ct].dma_start(
                                z[
                                    ic * 128 : (ic + 1) * 128,
                                    ct * 512 : (ct + 1) * 512,
                                ],
                                zr[:],
                            )

    nc.compile()
    _CACHE["nc"] = nc
    return nc


def _in_maps(x, Wq, Wkv, Wproj):
    cosT, sinNT, ones, ident, tri = _host_tables()
    s = 1.0 / np.sqrt(HD)
    xTs = [np.ascontiguousarray(x[b].T).astype(BF16) for b in range(B)]
    maps = []
    for core in range(NCORES):
        b, g = divmod(core, NKV)
        maps.append(
            {
                "xT": xTs[b],
                "wq": np.ascontiguousarray(Wq[:, g * GW : (g + 1) * GW] * s).astype(
                    BF16
                ),
                # pre-shuffled to SBUF layout [p, co, d]
                "wk": np.ascontiguousarray(
                    Wkv[:, g * HD : (g + 1) * HD]
                    .reshape(NCC, 128, HD)
                    .transpose(1, 0, 2)
                ).astype(BF16),
                "wv": np.ascontiguousarray(
                    Wkv[:, NKV * HD + g * HD : NKV * HD + (g + 1) * HD]
                    .reshape(NCC, 128, HD)
                    .transpose(1, 0, 2)
                ).astype(BF16),
                "wp": np.ascontiguousarray(Wproj[g * GW : (g + 1) * GW, :]).astype(
                    BF16
                ),
                "cosT": cosT,
                "sinNT": sinNT,
                "identm": ident,
                "onesm": ones,
                "trim": tri,
            }
        )
    return maps


def _run(inputs, trace=False, trace_kwargs=None):
    from concourse.bass_utils import run_bass_kernel_spmd

    nc = _build_nc()
    maps = _in_maps(
        np.asarray(inputs["x"], dtype=np.float32),
        np.asarray(inputs["Wq"], dtype=np.float32),
        np.asarray(inputs["Wkv"], dtype=np.float32),
        np.asarray(inputs["Wproj"], dtype=np.float32),
    )
    res = run_bass_kernel_spmd(
        nc, maps, list(range(NCORES)), trace=trace, **(trace_kwargs or {})
    )
    out = np.zeros((B, T, C), dtype=np.float32)
    for core in range(NCORES):
        b = core // NKV
        out[b] += np.asarray(res.results[core]["z"], dtype=np.float32)
    return out, res


def kernel(x, Wq, Wkv, Wproj):
    out, _ = _run({"x": x, "Wq": Wq, "Wkv": Wkv, "Wproj": Wproj}, trace=False)
    return out
